# revision 27
# baseline (speedup 1.0000x reference)
"""Trainium2 Bass kernel for nn_AGCNN (graph-attention point-cloud net).

Self-contained: builds an 8-core SPMD Bass/Tile graph, shards batch B=16
as 2 per core, runs via bass_utils.run_bass_kernel_spmd, reassembles.

Per batch element, per gsa layer (D = in channels):
  halfdist = X^T X - xx/2 (row) - xx/2 (col)      [PE matmul, augmented rows]
  top-20/21 per row: 16 contiguous-block max8 + merge rounds (max8 +
  match_replace) -> tmid = (v20+v21)/2, Z = sum exp(2*top20)
  sweep 2 (transposed Gram recompute): W^T = exp(2*halfdist) * (halfdist>=tmid)
  aggT = X @ W^T / Z      [PE matmul accumulating over column tiles]
  y = (Wc1 - Wc2) @ X + Wc2 @ aggT                [center-subtract folded]
  BN batch stats via AllReduce over the 8 cores, then scale/bias + LeakyReLU.
"""

import numpy as np
from contextlib import ExitStack

import concourse.bass as bass
import concourse.bacc as bacc
import concourse.mybir as mybir
import concourse.tile as tile
from concourse.bass_utils import run_bass_kernel_spmd

F32 = mybir.dt.float32
F32R = mybir.dt.float32r
AF = mybir.ActivationFunctionType
ALU = mybir.AluOpType

B, N = 16, 2048
EPS = 1e-5
NEG_SLOPE = 0.01
P = 128
NT = N // P          # 16
NHALF = N // 2       # 1024
BLOCAL = 2

GSA_LAYERS = [(3, 64), (64, 64), (64, 128), (128, 256)]
O5 = 512
NEG_BIG = -1.0e30


def build(n_cores: int, n_layers: int = 5):
    COUNT = float(n_cores * BLOCAL * N)
    nc = bacc.Bacc("TRN2", target_bir_lowering=False, debug=False)

    # ---------------- DRAM parameters ----------------
    x_in = nc.dram_tensor("x", [BLOCAL, 3, N], F32, kind="ExternalInput")
    ident_in = nc.dram_tensor("ident", [P, P], F32, kind="ExternalInput")
    wa_in, wb_in, gcol_in, bcol_in = {}, {}, {}, {}
    for li, (D, O) in enumerate(GSA_LAYERS):
        noc = max(1, O // P)
        wa_in[li] = nc.dram_tensor(f"wa{li}", [D, O], F32, kind="ExternalInput")
        wb_in[li] = nc.dram_tensor(f"wb{li}", [D, O],
                                   F32R if li >= 1 else F32,
                                   kind="ExternalInput")
        gcol_in[li] = nc.dram_tensor(f"gc{li}", [P, noc], F32, kind="ExternalInput")
        bcol_in[li] = nc.dram_tensor(f"bc{li}", [P, noc], F32, kind="ExternalInput")
    w5_in = nc.dram_tensor("w5t", [P, 5, O5], F32R, kind="ExternalInput")
    gcol_in[4] = nc.dram_tensor("gc4", [P, O5 // P], F32, kind="ExternalInput")
    bcol_in[4] = nc.dram_tensor("bc4", [P, O5 // P], F32, kind="ExternalInput")

    out5 = nc.dram_tensor("out5", [BLOCAL, O5, N], F32, kind="ExternalOutput")
    out3 = nc.dram_tensor("out3", [BLOCAL, P, N], F32, kind="ExternalOutput")

    replica_groups = [list(range(n_cores))]

    with tile.TileContext(nc) as tc, ExitStack() as ctx:
        sb_w = ctx.enter_context(tc.tile_pool(name="weights", bufs=1))
        sb_xa = ctx.enter_context(tc.tile_pool(name="xa", bufs=2))
        sb_xt = ctx.enter_context(tc.tile_pool(name="xtp", bufs=2))
        sb_big = ctx.enter_context(tc.tile_pool(name="bigp", bufs=3))
        sb_bc = ctx.enter_context(tc.tile_pool(name="bcp", bufs=2))
        sb_row = ctx.enter_context(tc.tile_pool(name="rowp", bufs=2))
        sb_sm = ctx.enter_context(tc.tile_pool(name="smallp", bufs=3))
        ps_g = ctx.enter_context(tc.tile_pool(name="psg", bufs=2, space="PSUM"))
        ps_a = ctx.enter_context(tc.tile_pool(name="psa", bufs=1, space="PSUM"))
        dram = ctx.enter_context(tc.tile_pool(name="dramp", bufs=1, space="DRAM"))

        # ---------------- constants / weights ----------------
        ident = sb_w.tile([P, P], F32, tag="ident")
        nc.sync.dma_start(ident[:], ident_in.ap())
        eps_t = sb_w.tile([P, 1], F32, tag="epst")
        nc.vector.memset(eps_t[:], EPS)
        ones_row = sb_w.tile([1, N], F32, tag="onesrow")
        nc.vector.memset(ones_row[:], 1.0)
        wa, wb, gcol, bcol = {}, {}, {}, {}
        for li, (D, O) in enumerate(GSA_LAYERS):
            noc = max(1, O // P)
            wa[li] = sb_w.tile([D, O], F32, tag=f"wa{li}", name=f"wa{li}")
            nc.sync.dma_start(wa[li][:], wa_in[li].ap())
            wb[li] = sb_w.tile([D, O], F32R if li >= 1 else F32,
                               tag=f"wb{li}", name=f"wb{li}")
            nc.sync.dma_start(wb[li][:], wb_in[li].ap())
            gcol[li] = sb_w.tile([P, noc], F32, tag=f"gc{li}", name=f"gc{li}")
            nc.sync.dma_start(gcol[li][:], gcol_in[li].ap())
            bcol[li] = sb_w.tile([P, noc], F32, tag=f"bc{li}", name=f"bc{li}")
            nc.sync.dma_start(bcol[li][:], bcol_in[li].ap())
        w5 = sb_w.tile([P, 5, O5], F32R, tag="w5t")
        nc.sync.dma_start(w5[:], w5_in.ap())
        gcol[4] = sb_w.tile([P, O5 // P], F32, tag="gc4", name="gc4")
        nc.sync.dma_start(gcol[4][:], gcol_in[4].ap())
        bcol[4] = sb_w.tile([P, O5 // P], F32, tag="bc4", name="bc4")
        nc.sync.dma_start(bcol[4][:], bcol_in[4].ap())

        # ---------------- DRAM intermediates ----------------
        y_dram = [dram.tile([O5, N], F32, tag=f"ydram{b}", name=f"ydram{b}") for b in range(BLOCAL)]
        xf_dram = {}
        for li, (D, O) in enumerate(GSA_LAYERS):
            for b in range(BLOCAL):
                xf_dram[(li, b)] = dram.tile([O, N], F32R, tag=f"xf{li}b{b}", name=f"xf{li}b{b}")
        cc_in, cc_out = {}, {}
        for li in range(5):
            O = GSA_LAYERS[li][1] if li < 4 else O5
            noc = max(1, O // P)
            cc_in[li] = dram.tile([P, 2 * noc], F32, tag=f"ccin{li}", name=f"ccin{li}")
            cc_out[li] = nc.dram_tensor(
                f"ccout{li}", [P, 2 * noc], F32,
                addr_space="Shared" if n_cores > 4 else "Local")

        XA1 = [None, None]
        XA2 = [None, None]
        AUX1 = [None, None]
        AUX2 = [None, None]
        XT = [None, None]
        XXN = [None, None]

        def alloc_xa(D):
            Da = max(32, -(-D // 32) * 32)
            if Da + 2 <= P:
                a1 = sb_xa.tile([Da + 2, N], F32, tag="xa1")
                a2 = sb_xa.tile([Da + 2, N], F32, tag="xa2")
                x1, x2 = None, None
            else:
                a1 = sb_xa.tile([Da, N], F32, tag="xa1")
                a2 = sb_xa.tile([Da, N], F32, tag="xa2")
                x1 = sb_xt.tile([2, N], F32, tag="aux1", bufs=2)
                x2 = sb_xt.tile([2, N], F32, tag="aux2", bufs=2)
            return a1, a2, x1, x2

        def build_aux_rows(b, D):
            xa1, xa2 = XA1[b], XA2[b]
            tp = ps_g.tile([NT, P], F32, tag="gram")
            nc.tensor.transpose(tp[:], XXN[b][:], ident[:])
            row16 = sb_sm.tile([NT, P], F32, tag="row16")
            nc.scalar.activation(row16[:], tp[:], AF.Copy)
            Da = max(32, -(-D // 32) * 32)
            if Da + 2 <= P:
                nc.vector.memset(xa1[Da : Da + 1, :], 1.0)
                nc.sync.dma_start(xa2[Da + 1 : Da + 2, :], ones_row[:])
                nc.sync.dma_start(xa1[Da + 1 : Da + 2, :], row16[:])
                nc.sync.dma_start(xa2[Da : Da + 1, :], row16[:])
            else:
                aux1, aux2 = AUX1[b], AUX2[b]
                nc.vector.memset(aux1[0:1, :], 1.0)
                nc.sync.dma_start(aux2[1:2, :], ones_row[:])
                nc.sync.dma_start(aux1[1:2, :], row16[:])
                nc.sync.dma_start(aux2[0:1, :], row16[:])

        def build_xt_xx(b, D, dt_agg):
            xt = sb_xt.tile([P, NT, P], dt_agg, tag="xt")
            XT[b] = xt
            xxc = sb_sm.tile([P, NT], F32, tag="xxc")
            sqj = sb_sm.tile([P, P], F32, tag="yjunk")
            for c in range(NT):
                tp = ps_g.tile([P, D], F32, tag="gram")
                nc.tensor.transpose(tp[:], XA1[b][0:D, c * P : (c + 1) * P],
                                    ident[0:D, 0:D])
                nc.scalar.activation(xt[:, c, 0:D], tp[:], AF.Copy)
                nc.scalar.activation(sqj[:, 0:D], xt[:, c, 0:D], AF.Square,
                                     accum_out=xxc[:, c : c + 1])
            xxn = sb_sm.tile([P, NT], F32, tag="xxn")
            nc.vector.tensor_scalar(xxn[:], xxc[:], -0.5, None, op0=ALU.mult)
            XXN[b] = xxn

        # ---------------- L1 input prep ----------------
        for b in range(BLOCAL):
            D = 3
            xa1, xa2, ax1, ax2 = alloc_xa(D)
            XA1[b], XA2[b] = xa1, xa2
            AUX1[b], AUX2[b] = ax1, ax2
            nc.vector.memset(xa1[0:32, :], 0.0)
            nc.vector.memset(xa2[0:32, :], 0.0)
            nc.sync.dma_start(xa1[0:D, :], x_in.ap()[b])
            nc.vector.tensor_copy(xa2[0:D, :], xa1[0:D, :])
            build_xt_xx(b, D, F32)
            build_aux_rows(b, D)

        def bn_allreduce_and_coeffs(li, O, sparts):
            """Reduce per-b partial sums, AllReduce, return (aa, bb) tiles."""
            noc = max(1, O // P)
            ccs = sb_sm.tile([P, 2 * noc], F32, tag="ccs")
            for oc in range(noc):
                for q in range(2):
                    tmp = sb_sm.tile([P, 2 * BLOCAL * 4], F32, tag="cctmp")
                    for b in range(BLOCAL):
                        v = sparts[b][:].rearrange("p (c t) -> p c t", t=2)[
                            :, oc * 4 : oc * 4 + 4, q]
                        nc.vector.tensor_copy(tmp[:, b * 4 : b * 4 + 4], v)
                    nc.vector.reduce_sum(ccs[:, oc * 2 + q : oc * 2 + q + 1],
                                         tmp[:, 0 : 4 * BLOCAL],
                                         axis=mybir.AxisListType.X)
            nc.sync.dma_start(cc_in[li][:], ccs[:])
            nc.gpsimd.collective_compute(
                "AllReduce", ALU.add,
                ins=[cc_in[li][:].opt()],
                outs=[cc_out[li].ap().opt()],
                replica_groups=replica_groups,
            )
            cco = sb_sm.tile([P, 2 * noc], F32, tag="cco")
            nc.sync.dma_start(cco[:], cc_out[li].ap())
            mean = sb_sm.tile([P, noc], F32, tag="mean")
            var = sb_sm.tile([P, noc], F32, tag="var")
            av = cco[:].rearrange("p (c t) -> p c t", t=2)
            nc.vector.tensor_scalar(mean[:], av[:, :, 0], 1.0 / COUNT, None,
                                    op0=ALU.mult)
            nc.vector.tensor_scalar(var[:], av[:, :, 1], 1.0 / COUNT, None,
                                    op0=ALU.mult)
            m2 = sb_sm.tile([P, noc], F32, tag="m2")
            nc.vector.tensor_tensor(m2[:], mean[:], mean[:], ALU.mult)
            nc.vector.tensor_tensor(var[:], var[:], m2[:], ALU.subtract)
            sd = sb_sm.tile([P, noc], F32, tag="sd")
            nc.scalar.activation(sd[:], var[:], AF.Sqrt, bias=eps_t[:])
            rsd = sb_sm.tile([P, noc], F32, tag="rsd")
            nc.vector.reciprocal(rsd[:], sd[:])
            aa = sb_sm.tile([P, noc], F32, tag="aa")
            nc.vector.tensor_tensor(aa[:], gcol[li][:], rsd[:], ALU.mult)
            bb = sb_sm.tile([P, noc], F32, tag="bb")
            nc.vector.tensor_tensor(bb[:], mean[:], aa[:], ALU.mult)
            nc.vector.tensor_tensor(bb[:], bcol[li][:], bb[:], ALU.subtract)
            return aa, bb

        # ---------------- gsa layers ----------------
        def gsa_layer(li, D, O):
            sparts = [None, None]

            for b in range(BLOCAL):
                xa1, xa2 = XA1[b], XA2[b]
                Da = max(32, -(-D // 32) * 32)
                if Da + 2 <= P:
                    gops = [(xa1, xa2, 0, Da + 2)]
                else:
                    gops = [(xa1, xa2, 0, Da),
                            (AUX1[b], AUX2[b], 0, 2)]
                tmc = sb_sm.tile([P, NT], F32, tag=f"tmc{b}")
                zc = sb_sm.tile([P, NT], F32, tag=f"zc{b}")

                # ---------- sweep 1 ----------
                for i in range(NT):
                    dist = sb_big.tile([P, N], F32, tag="big")
                    for h in range(2):
                        g = ps_g.tile([P, NHALF], F32, tag="gram")
                        for s in range(2):
                            sl = slice(h * NHALF + s * 512,
                                       h * NHALF + (s + 1) * 512)
                            for ki, (lt, rt, k0, k1) in enumerate(gops):
                                nc.tensor.matmul(
                                    g[:, s * 512 : (s + 1) * 512],
                                    lt[k0:k1, i * P : (i + 1) * P],
                                    rt[k0:k1, sl],
                                    start=(ki == 0),
                                    stop=(ki == len(gops) - 1),
                                )
                        nc.scalar.activation(
                            dist[:, h * NHALF : (h + 1) * NHALF], g[:], AF.Copy)
                    cand = sb_sm.tile([P, 16 * 8], F32, tag="cand")
                    for gi in range(16):
                        nc.vector.max(cand[:, gi * 8 : (gi + 1) * 8],
                                      dist[:, gi * P : (gi + 1) * P])
                    t8 = sb_sm.tile([P, 24], F32, tag="t8")
                    s1 = sb_sm.tile([P, 128], F32, tag="s1")
                    s2 = sb_sm.tile([P, 128], F32, tag="s2")
                    nc.vector.max(t8[:, 0:8], cand[:])
                    nc.vector.match_replace(s1[:], t8[:, 0:8], cand[:], NEG_BIG)
                    nc.vector.max(t8[:, 8:16], s1[:])
                    nc.vector.match_replace(s2[:], t8[:, 8:16], s1[:], NEG_BIG)
                    nc.vector.max(t8[:, 16:24], s2[:])
                    nc.vector.tensor_tensor(tmc[:, i : i + 1], t8[:, 19:20],
                                            t8[:, 20:21], ALU.add)
                    nc.vector.tensor_scalar(tmc[:, i : i + 1],
                                            tmc[:, i : i + 1], 0.5, None,
                                            op0=ALU.mult)
                    ez = sb_sm.tile([P, 20], F32, tag="ez")
                    nc.scalar.activation(ez[:], t8[:, 0:20], AF.Exp, scale=2.0)
                    nc.vector.reduce_sum(zc[:, i : i + 1], ez[:],
                                         axis=mybir.AxisListType.X)

                # ---------- mid: broadcasts ----------
                rz = sb_sm.tile([P, NT], F32, tag="rzc")
                nc.vector.reciprocal(rz[:], zc[:])
                tmbc = sb_bc.tile([P, N], F32, tag="bc")
                rzbc = sb_bc.tile([P, N], F32, tag="bc")
                for src, dst in ((tmc, tmbc), (rz, rzbc)):
                    tp = ps_g.tile([NT, P], F32, tag="gram")
                    nc.tensor.transpose(tp[:], src[:], ident[:])
                    r16 = sb_sm.tile([NT, P], F32, tag="row16")
                    nc.scalar.activation(r16[:], tp[:], AF.Copy)
                    row = sb_row.tile([1, N], F32, tag="row1")
                    nc.sync.dma_start(row[:], r16[:])
                    bcps = ps_a.tile([P, N], F32, tag="agg", name="bcps")
                    for s in range(4):
                        nc.tensor.matmul(bcps[:, s * 512 : (s + 1) * 512],
                                         ones_row[0:1, 0:P],
                                         row[0:1, s * 512 : (s + 1) * 512],
                                         start=True, stop=True)
                    nc.scalar.activation(dst[:], bcps[:], AF.Copy)

                # ---------- sweep 2 ----------
                dt_agg = F32R if li >= 1 else F32
                aggp = ps_a.tile([P, N], F32, tag="agg")
                for j in range(NT):
                    ft = sb_big.tile([P, N], F32, tag="big")
                    m01 = sb_big.tile([P, N], dt_agg, tag="big")
                    for h in range(2):
                        g = ps_g.tile([P, NHALF], F32, tag="gram")
                        for s in range(2):
                            sl = slice(h * NHALF + s * 512,
                                       h * NHALF + (s + 1) * 512)
                            for ki, (lt, rt, k0, k1) in enumerate(gops):
                                nc.tensor.matmul(
                                    g[:, s * 512 : (s + 1) * 512],
                                    lt[k0:k1, j * P : (j + 1) * P],
                                    rt[k0:k1, sl],
                                    start=(ki == 0),
                                    stop=(ki == len(gops) - 1),
                                )
                        hs = slice(h * NHALF, (h + 1) * NHALF)
                        nc.scalar.activation(ft[:, hs], g[:], AF.Exp, scale=2.0)
                        nc.vector.tensor_tensor(m01[:, hs], g[:], tmbc[:, hs],
                                                ALU.is_ge)
                    nc.vector.tensor_tensor(m01[:], ft[:], m01[:], ALU.mult)
                    for s in range(4):
                        nc.tensor.matmul(
                            aggp[0:D, s * 512 : (s + 1) * 512],
                            XT[b][:, j, 0:D],
                            m01[:, s * 512 : (s + 1) * 512],
                            start=(j == 0),
                            stop=(j == NT - 1),
                        )

                aggt = sb_big.tile([P, N], dt_agg, tag="big")
                nc.vector.tensor_tensor(aggt[0:D, :], aggp[0:D, :],
                                        rzbc[0:D, :], ALU.mult)

                # ---------- conv + raw stats ----------
                noc = max(1, O // P)
                osz = min(P, O)
                spart = sb_sm.tile([P, noc * 4 * 2], F32, tag=f"spart{b}")
                sparts[b] = spart
                if osz < P:
                    nc.vector.memset(spart[:], 0.0)
                for oc in range(noc):
                    for s in range(4):
                        yp = ps_g.tile([P, 512], F32, tag="gram")
                        nc.tensor.matmul(yp[0:osz, :],
                                         wa[li][:, oc * P : oc * P + osz],
                                         xa1[0:D, s * 512 : (s + 1) * 512],
                                         start=True, stop=False)
                        nc.tensor.matmul(yp[0:osz, :],
                                         wb[li][:, oc * P : oc * P + osz],
                                         aggt[0:D, s * 512 : (s + 1) * 512],
                                         start=False, stop=True)
                        ysb = sb_sm.tile([P, 512], F32, tag="ysb")
                        ci = (oc * 4 + s) * 2
                        nc.scalar.activation(ysb[0:osz, :], yp[0:osz, :],
                                             AF.Copy,
                                             accum_out=spart[0:osz, ci : ci + 1])
                        yjunk = sb_sm.tile([P, 512], F32, tag="yjunk")
                        nc.scalar.activation(yjunk[0:osz, :], yp[0:osz, :],
                                             AF.Square,
                                             accum_out=spart[0:osz, ci + 1 : ci + 2])
                        nc.sync.dma_start(
                            y_dram[b][:][oc * P : oc * P + osz,
                                         s * 512 : (s + 1) * 512],
                            ysb[0:osz, :])

            aa, bb = bn_allreduce_and_coeffs(li, O, sparts)

            # ---------- apply + next-layer state ----------
            is_last_gsa = (li == 3)
            Dn = O
            noc = max(1, O // P)
            osz = min(P, O)
            for b in range(BLOCAL):
                if not is_last_gsa:
                    xa1n, xa2n, ax1n, ax2n = alloc_xa(Dn)
                for oc in range(noc):
                    for s in range(4):
                        yt = sb_sm.tile([P, 512], F32, tag="ysb")
                        nc.sync.dma_start(
                            yt[0:osz, :],
                            y_dram[b][:][oc * P : oc * P + osz,
                                         s * 512 : (s + 1) * 512])
                        z = sb_sm.tile([P, 512], F32, tag="zt")
                        nc.scalar.activation(z[0:osz, :], yt[0:osz, :],
                                             AF.Identity,
                                             bias=bb[0:osz, oc : oc + 1],
                                             scale=aa[0:osz, oc : oc + 1])
                        z2 = sb_sm.tile([P, 512], F32, tag="z2t")
                        nc.scalar.activation(z2[0:osz, :], z[0:osz, :], AF.Copy,
                                             scale=NEG_SLOPE)
                        if is_last_gsa:
                            xo = sb_sm.tile([P, 512], F32R, tag="xo")
                            nc.vector.tensor_tensor(xo[0:osz, :], z[0:osz, :],
                                                    z2[0:osz, :], ALU.max)
                            nc.sync.dma_start(
                                xf_dram[(li, b)][:][oc * P : oc * P + osz,
                                                    s * 512 : (s + 1) * 512],
                                xo[0:osz, :])
                        else:
                            nc.vector.tensor_tensor(
                                xa1n[oc * P : oc * P + osz,
                                     s * 512 : (s + 1) * 512],
                                z[0:osz, :], z2[0:osz, :], ALU.max)
                if not is_last_gsa:
                    nc.vector.tensor_copy(xa2n[0:Dn, :], xa1n[0:Dn, :])
                    xsp = sb_big.tile([P, N], F32R, tag="big", name="xsp")
                    nc.vector.tensor_copy(xsp[0:Dn, :], xa1n[0:Dn, :])
                    nc.sync.dma_start(xf_dram[(li, b)][:], xsp[0:Dn, :])
                    if li == 2:
                        nc.sync.dma_start(out3.ap()[b], xa1n[0:Dn, :])
                    XA1[b], XA2[b] = xa1n, xa2n
                    AUX1[b], AUX2[b] = ax1n, ax2n
                    build_xt_xx(b, Dn, F32R)
                    build_aux_rows(b, Dn)

        for li, (D, O) in enumerate(GSA_LAYERS):
            if li < n_layers:
                gsa_layer(li, D, O)

        if n_layers < 5:
            # debug: dump first available feature to outputs and stop
            for b in range(BLOCAL):
                lj = max(0, n_layers - 1)
                Od = GSA_LAYERS[lj][1]
                tdump = sb_sm.tile([P, 512], F32, tag="ysb", name="tdump")
                for oc in range(max(1, Od // P)):
                    osz2 = min(P, Od)
                    for s in range(4):
                        nc.sync.dma_start(
                            tdump[0:osz2, :],
                            xf_dram[(lj, b)][:][oc * P : oc * P + osz2,
                                                s * 512 : (s + 1) * 512])
                        nc.sync.dma_start(
                            out3.ap()[b][oc * P : oc * P + osz2,
                                         s * 512 : (s + 1) * 512]
                            if oc == 0 else
                            out5.ap()[b][(oc - 1) * P : (oc - 1) * P + osz2,
                                         s * 512 : (s + 1) * 512],
                            tdump[0:osz2, :])
            _finish_debug = True
        else:
            _finish_debug = False

        # ---------------- L5 ----------------
        if _finish_debug:
            kparts = []

        li5 = 4
        noc5 = O5 // P
        off = 0
        for lj, (Dj, Oj) in enumerate(GSA_LAYERS):
            for r0 in range(0, Oj, P):
                kparts.append((lj, r0, min(P, Oj - r0), off))
                off += min(P, Oj - r0)
        assert off == O5

        sparts5 = [None, None]
        for b in range(BLOCAL):
            spart = sb_sm.tile([P, noc5 * 4 * 2], F32, tag=f"spart{b}")
            sparts5[b] = spart
            for oc in range(noc5):
                for s in range(4):
                    yp = ps_g.tile([P, 512], F32, tag="gram")
                    for ki, (lj, r0, rows, woff) in enumerate(kparts):
                        base = woff % P
                        xk = sb_sm.tile([P, 512], F32R, tag="xk")
                        nc.sync.dma_start(
                            xk[base : base + rows, :],
                            xf_dram[(lj, b)][:][r0 : r0 + rows,
                                                s * 512 : (s + 1) * 512])
                        nc.tensor.matmul(
                            yp[:],
                            w5[base : base + rows, woff // P,
                               oc * P : (oc + 1) * P],
                            xk[base : base + rows, :],
                            start=(ki == 0),
                            stop=(ki == len(kparts) - 1))
                    ysb = sb_sm.tile([P, 512], F32, tag="ysb")
                    ci = (oc * 4 + s) * 2
                    nc.scalar.activation(ysb[:], yp[:], AF.Copy,
                                         accum_out=spart[:, ci : ci + 1])
                    yjunk = sb_sm.tile([P, 512], F32, tag="yjunk")
                    nc.scalar.activation(yjunk[:], yp[:], AF.Square,
                                         accum_out=spart[:, ci + 1 : ci + 2])
                    nc.sync.dma_start(
                        y_dram[b][:][oc * P : (oc + 1) * P,
                                     s * 512 : (s + 1) * 512],
                        ysb[:])

        aa, bb = bn_allreduce_and_coeffs(li5, O5, sparts5)
        for b in range(BLOCAL):
            for oc in range(noc5):
                for s in range(4):
                    yt = sb_sm.tile([P, 512], F32, tag="ysb")
                    nc.sync.dma_start(
                        yt[:], y_dram[b][:][oc * P : (oc + 1) * P,
                                            s * 512 : (s + 1) * 512])
                    z = sb_sm.tile([P, 512], F32, tag="zt")
                    nc.scalar.activation(z[:], yt[:], AF.Identity,
                                         bias=bb[:, oc : oc + 1],
                                         scale=aa[:, oc : oc + 1])
                    z2 = sb_sm.tile([P, 512], F32, tag="z2t")
                    nc.scalar.activation(z2[:], z[:], AF.Copy, scale=NEG_SLOPE)
                    xo = sb_sm.tile([P, 512], F32, tag="xo")
                    nc.vector.tensor_tensor(xo[:], z[:], z2[:], ALU.max)
                    nc.sync.dma_start(
                        out5.ap()[b][oc * P : (oc + 1) * P,
                                     s * 512 : (s + 1) * 512],
                        xo[:])

    nc.compile()
    return nc


def make_in_maps(inputs, n_cores):
    x = np.ascontiguousarray(np.asarray(inputs["x"], np.float32))
    Ws = [np.asarray(inputs[f"W{i}"], np.float32) for i in range(1, 6)]
    gs = [np.asarray(inputs[f"g{i}"], np.float32) for i in range(1, 6)]
    bs = [np.asarray(inputs[f"b{i}"], np.float32) for i in range(1, 6)]

    def col_layout(v, O):
        noc = max(1, O // P)
        out = np.zeros((P, noc), np.float32)
        for oc in range(noc):
            rows = min(P, O - oc * P)
            out[:rows, oc] = v[oc * P : oc * P + rows]
        return out

    common = {"ident": np.eye(P, dtype=np.float32)}
    for li, (D, O) in enumerate(GSA_LAYERS):
        W = Ws[li]
        Wc1, Wc2 = W[:, :D], W[:, D:]
        common[f"wa{li}"] = np.ascontiguousarray((Wc1 - Wc2).T)
        common[f"wb{li}"] = np.ascontiguousarray(Wc2.T)
        common[f"gc{li}"] = col_layout(gs[li], O)
        common[f"bc{li}"] = col_layout(bs[li], O)
    w5t = Ws[4].T  # (512 rows = in-chan, 512 cols = out-chan)
    w5p = np.zeros((P, 5, O5), np.float32)
    woff = 0
    for ki, rows in enumerate([64, 64, 128, 128, 128]):
        w5p[0:rows, ki, :] = w5t[woff : woff + rows, :]
        woff += rows
    common["w5t"] = w5p
    common["gc4"] = col_layout(gs[4], O5)
    common["bc4"] = col_layout(bs[4], O5)

    in_maps = []
    for c in range(n_cores):
        m = dict(common)
        m["x"] = np.ascontiguousarray(x[c * BLOCAL : (c + 1) * BLOCAL])
        in_maps.append(m)
    return in_maps


_BUILD_CACHE = {}


def _get_nc(n_cores):
    if n_cores not in _BUILD_CACHE:
        _BUILD_CACHE[n_cores] = build(n_cores)
    return _BUILD_CACHE[n_cores]


def kernel(**inputs):
    n_cores = 8
    nc = _get_nc(n_cores)
    in_maps = make_in_maps(inputs, n_cores)
    res = run_bass_kernel_spmd(nc, in_maps, list(range(n_cores)))
    x5 = np.concatenate([res.results[c]["out5"] for c in range(n_cores)], axis=0)
    x3 = np.concatenate([res.results[c]["out3"] for c in range(n_cores)], axis=0)
    return x5.astype(np.float32), x3.astype(np.float32)


if __name__ == "__main__":
    nc = build(8)
    print("build ok:", len(nc.inst_map), "instructions")
